# revision 34
# baseline (speedup 1.0000x reference)
"""AdaptiveCurvatureLoss on 8 TRN2 NeuronCores — bitonic-sort kNN variant.

The kNN density of a 1-D point set needs only the sorted order: each point's
two nearest neighbours lie within +-2 positions in sorted order.  So instead
of the O(N^2) pairwise matrix, every core sorts the full x (16384 values as a
[128, 128] tile) with a bitonic network:
  - compare-exchange stages along the free dim (pairs at distance j) as
    tensor_tensor min/max over strided views,
  - descending blocks handled by per-partition sign flips (host-supplied
    masks) for k >= 128, and by separate asc/desc views for k <= 64,
  - pair distances >= 128 via PE transpose (work in transposed index space).
Then neighbour diffs + a 4-candidate window give exact f32 densities.
The MLP / second-derivative / MSE parts stay row-sharded across the 8 cores
as before; host combines partial sums (scalar epilogue only).
"""

import sys

sys.path.insert(0, "/opt/trn_rl_repo")

import numpy as np

import concourse.mybir as mybir
from concourse import bacc
from concourse.bass_utils import run_bass_kernel_spmd
from concourse.tile import TileContext

N = 16384
NCORES = 8
SHARD = N // NCORES          # 2048
P = 128
W = 128                      # sort grid: [128 partitions, 128 free]
NCH = SHARD // P             # 16
H = 64
EPS = 1e-8
BIG = 1e30
F32 = mybir.dt.float32
ALU = mybir.AluOpType
ACTF = mybir.ActivationFunctionType

SGN_KS = [128 << t for t in range(7)]  # 128..8192


def _build():
    nc = bacc.Bacc()
    x_full = nc.declare_dram_parameter("x_full", [N], F32, isOutput=False)
    t_sh = nc.declare_dram_parameter("t_shard", [P, NCH], F32, isOutput=False)
    w1 = nc.declare_dram_parameter("w1", [H], F32, isOutput=False)
    b1 = nc.declare_dram_parameter("b1", [H], F32, isOutput=False)
    w2 = nc.declare_dram_parameter("w2", [H], F32, isOutput=False)
    b2 = nc.declare_dram_parameter("b2", [1], F32, isOutput=False)
    c2n = nc.declare_dram_parameter("c2n", [H], F32, isOutput=False)  # 2*w1^2*w2
    xf = nc.declare_dram_parameter("x_flat", [1, 2 * SHARD], F32, isOutput=False)  # [x | ones]
    sgn = nc.declare_dram_parameter("signs", [P, len(SGN_KS)], F32, isOutput=False)
    idn = nc.declare_dram_parameter("ident", [P, P], F32, isOutput=False)
    shu = nc.declare_dram_parameter("shiftu", [P, P], F32, isOutput=False)
    shd = nc.declare_dram_parameter("shiftd", [P, P], F32, isOutput=False)
    out = nc.declare_dram_parameter("out", [P, W + 2], F32, isOutput=True)

    with TileContext(nc) as tc:
        with (
            tc.tile_pool(name="sp", bufs=1) as sp,
            tc.tile_pool(name="ps", bufs=2, space="PSUM") as ps,
            tc.tile_pool(name="ups", bufs=4, space="PSUM") as upsp,
        ):
            # ---- loads ----
            sortA = sp.tile([P, W], F32)
            nc.sync.dma_start(sortA[:, :], x_full.ap().rearrange("(p f) -> p f", p=P))
            sortB = sp.tile([P, W], F32)
            signs = sp.tile([P, len(SGN_KS)], F32)
            nc.sync.dma_start(signs[:, :], sgn[:, :])
            ident = sp.tile([P, P], F32)
            nc.sync.dma_start(ident[:, :], idn[:, :])
            # non-sort-critical loads go on the gpsimd DMA queue so they don't
            # serialize behind the sort's sync-queue DMAs
            shiftu = sp.tile([P, P], F32)
            nc.gpsimd.dma_start(shiftu[:, :], shu[:, :])
            shiftd = sp.tile([P, P], F32)
            nc.gpsimd.dma_start(shiftd[:, :], shd[:, :])
            tsh = sp.tile([P, NCH], F32)
            nc.gpsimd.dma_start(tsh[:, :], t_sh[:, :])
            w1r = sp.tile([P, H], F32)
            nc.gpsimd.dma_start(w1r[:, :], w1.ap().partition_broadcast(P))
            b1r = sp.tile([P, H], F32)
            nc.gpsimd.dma_start(b1r[:, :], b1.ap().partition_broadcast(P))
            w2r = sp.tile([P, H], F32)
            nc.gpsimd.dma_start(w2r[:, :], w2.ap().partition_broadcast(P))
            c2r = sp.tile([P, H], F32)
            nc.gpsimd.dma_start(c2r[:, :], c2n.ap().partition_broadcast(P))
            b2s = sp.tile([P, 1], F32)
            nc.gpsimd.dma_start(b2s[:, :], b2.ap().partition_broadcast(P))
            xfs = sp.tile([1, 2 * SHARD], F32)
            nc.gpsimd.dma_start(xfs[:, :], xf[:, :])
            out_sb = sp.tile([P, W + 2], F32)
            # BIG row for the auxU boundary, prepared up front
            auxU = sp.tile([P, 2], F32)
            bigc = sp.tile([P, 2], F32)
            nc.vector.memset(bigc[:, :], BIG)
            nc.gpsimd.dma_start(auxU[P - 1 : P, 0:2], bigc[0:1, 0:2])

            # ---- MLP / derivative / mse (sharded; overlaps the sort) ----
            def bc_h(ap2d):
                return ap2d.rearrange("p (o h) -> p o h", o=1).to_broadcast((P, NCH, H))

            u = sp.tile([P, NCH, H], F32)
            th = sp.tile([P, NCH, H], F32)
            g = sp.tile([P, NCH, H], F32)
            # u_c = x_c (x) w1 + 1 (x) b1 on the (otherwise idle) PE, tanh from PSUM
            for c in range(NCH):
                ups = upsp.tile([P, H], F32, tag="ups")
                cs = slice(c * P, (c + 1) * P)
                nc.tensor.matmul(ups[:, :], xfs[0:1, cs], w1r[0:1, :], start=True, stop=False)
                cso = slice(SHARD + c * P, SHARD + (c + 1) * P)
                nc.tensor.matmul(ups[:, :], xfs[0:1, cso], b1r[0:1, :], start=False, stop=True)
                nc.scalar.activation(th[:, c, :], ups[:, :], ACTF.Tanh)
            nc.scalar.activation(u[:, :, :], th[:, :, :], ACTF.Square)
            nc.vector.scalar_tensor_tensor(
                g[:, :, :], u[:, :, :], 1.0, th[:, :, :], op0=ALU.subtract, op1=ALU.mult
            )
            pred = sp.tile([P, NCH], F32)
            d2t = sp.tile([P, NCH], F32)
            nc.vector.tensor_tensor(u[:, :, :], th[:, :, :], bc_h(w2r[:, :]), op=ALU.mult)
            nc.vector.tensor_reduce(pred[:, :], u[:, :, :], axis=mybir.AxisListType.X, op=ALU.add)
            nc.vector.tensor_tensor(u[:, :, :], g[:, :, :], bc_h(c2r[:, :]), op=ALU.mult)
            nc.vector.tensor_reduce(d2t[:, :], u[:, :, :], axis=mybir.AxisListType.X, op=ALU.add)
            e = sp.tile([P, NCH], F32)
            esq = sp.tile([P, NCH], F32)
            nc.vector.scalar_tensor_tensor(
                e[:, :], pred[:, :], b2s[:, 0:1], tsh[:, :], op0=ALU.add, op1=ALU.subtract
            )
            nc.scalar.activation(
                esq[:, :], e[:, :], ACTF.Square, accum_out=out_sb[:, W : W + 1]
            )
            nc.scalar.activation(
                esq[:, :], d2t[:, :], ACTF.Square, accum_out=out_sb[:, W + 1 : W + 2]
            )

            # ---- bitonic sort ----
            def lo_hi_views(t, k, j):
                """(lo, hi, is_asc) view pairs of a [P, W] tile t for one stage."""
                if k >= W:
                    v = t[:, :].rearrange("p (c s) -> p c s", s=2 * j)
                    return [(v[:, :, 0:j], v[:, :, j : 2 * j], True)]
                v = t[:, :].rearrange("p (b r) -> p b r", r=2 * k)
                asc = v[:, :, 0:k].rearrange("p b (c s) -> p b c s", s=2 * j)
                desc = v[:, :, k : 2 * k].rearrange("p b (c s) -> p b c s", s=2 * j)
                return [
                    (asc[:, :, :, 0:j], asc[:, :, :, j : 2 * j], True),
                    (desc[:, :, :, 0:j], desc[:, :, :, j : 2 * j], False),
                ]

            # A phase's opening negate rides the post-transpose PSUM->SBUF
            # copy (ACT scale); its closing negate is a cheap in-place DVE
            # tensor_scalar.  Transposes use the fast PE is_transpose path.
            cur, alt = sortA, sortB

            def do_stage(k, j):
                nonlocal cur, alt
                for lo, hi, is_asc in lo_hi_views(cur, k, j):
                    alo, ahi, _ = lo_hi_views(alt, k, j)[0 if is_asc else 1]
                    nc.vector.tensor_tensor(alo, lo, hi, op=ALU.min if is_asc else ALU.max)
                    nc.vector.tensor_tensor(ahi, lo, hi, op=ALU.max if is_asc else ALU.min)
                cur, alt = alt, cur

            def do_transpose(scale_col):
                nonlocal cur, alt
                pt = ps.tile([P, W], F32, tag="tpsum")
                nc.tensor.transpose(pt[:, :], cur[:, :], ident[:, :])
                if scale_col is not None:
                    nc.scalar.mul(alt[:, :], pt[:, :], signs[:, scale_col : scale_col + 1])
                else:
                    nc.scalar.copy(alt[:, :], pt[:, :])
                cur, alt = alt, cur

            for t in range(1, 15):
                k = 1 << t
                js = [k >> s for s in range(1, 20) if (k >> s) >= 1]
                if k <= 64:
                    for j in js:
                        do_stage(k, j)
                    continue
                cross = [j for j in js if j >= W]
                if cross:
                    do_transpose(None)
                    for j in cross:
                        do_stage(min(k // W, W), j // W)
                    do_transpose(SGN_KS.index(k) if k < N else None)
                else:
                    # k == 128: opening negate as a standalone ACT scaled copy
                    nc.scalar.mul(alt[:, :], cur[:, :], signs[:, 0:1])
                    cur, alt = alt, cur
                for j in js:
                    if j < W:
                        do_stage(W, j)
                if 128 <= k < N:
                    # closing un-negate, in place on the DVE
                    col = SGN_KS.index(k)
                    nc.vector.tensor_scalar(
                        cur[:, :], cur[:, :], signs[:, col : col + 1], None, op0=ALU.mult
                    )

            s = cur  # sorted ascending, idx = p*W + f

            # ---- neighbour diffs + 4-candidate window ----
            # Row-boundary values via PE shift-matrices (no slow partition-
            # shift DMAs): auxU[p] = s[p+1, col], auxD[p] = s[p-1, col].
            pu = ps.tile([P, 2], F32, tag="shpsum")
            nc.tensor.matmul(pu[:, :], shiftu[:, :], s[:, 0:2])
            # partition 127 was pre-filled with BIG via DMA (engines can't
            # address a 1-partition range at p=127); copy only 0..126 here
            nc.scalar.copy(auxU[0 : P - 1, :], pu[0 : P - 1, :])
            pd = ps.tile([P, 2], F32, tag="shpsum")
            nc.tensor.matmul(pd[:, :], shiftd[:, :], s[:, W - 2 : W])
            auxD = sp.tile([P, 2], F32)
            nc.scalar.copy(auxD[:, :], pd[:, :])
            dR = sp.tile([P, W + 1], F32)   # col c: R1 at idx p*W + c - 1
            d2 = sp.tile([P, W + 2], F32)   # col c: R2 at idx p*W + c - 2
            nc.vector.tensor_sub(dR[:, 1:W], s[:, 1:W], s[:, 0 : W - 1])
            nc.vector.tensor_sub(dR[:, W : W + 1], auxU[:, 0:1], s[:, W - 1 : W])
            nc.vector.tensor_sub(dR[:, 0:1], s[:, 0:1], auxD[:, 1:2])
            nc.vector.memset(dR[0:1, 0:1], BIG)
            nc.vector.tensor_sub(d2[:, 2:W], s[:, 2:W], s[:, 0 : W - 2])
            nc.vector.tensor_sub(d2[:, W : W + 1], auxU[:, 0:1], s[:, W - 2 : W - 1])
            nc.vector.tensor_sub(d2[:, W + 1 : W + 2], auxU[:, 1:2], s[:, W - 1 : W])
            nc.vector.tensor_sub(d2[:, 0:2], s[:, 0:2], auxD[:, 0:2])
            nc.vector.memset(d2[0:1, 0:2], BIG)

            ca = dR[:, 1 : W + 1]   # R1
            cb = dR[:, 0:W]         # L1
            cc = d2[:, 2 : W + 2]   # R2
            cd = d2[:, 0:W]         # L2
            ab_lo = sp.tile([P, W], F32)
            ab_hi = sp.tile([P, W], F32)
            cd_lo = sp.tile([P, W], F32)
            cd_hi = sp.tile([P, W], F32)
            nc.vector.tensor_tensor(ab_lo[:, :], ca, cb, op=ALU.min)
            nc.vector.tensor_tensor(ab_hi[:, :], ca, cb, op=ALU.max)
            nc.vector.tensor_tensor(cd_lo[:, :], cc, cd, op=ALU.min)
            nc.vector.tensor_tensor(cd_hi[:, :], cc, cd, op=ALU.max)
            m1 = sp.tile([P, W], F32)
            mm = sp.tile([P, W], F32)
            nc.vector.tensor_tensor(m1[:, :], ab_lo[:, :], cd_lo[:, :], op=ALU.min)
            nc.vector.tensor_tensor(mm[:, :], ab_lo[:, :], cd_lo[:, :], op=ALU.max)
            nc.vector.tensor_tensor(ab_lo[:, :], ab_hi[:, :], cd_hi[:, :], op=ALU.min)
            nc.vector.tensor_tensor(mm[:, :], mm[:, :], ab_lo[:, :], op=ALU.min)
            # ship d1 + d2; the reciprocal/density happens on host
            nc.vector.tensor_add(out_sb[:, 0:W], m1[:, :], mm[:, :])

            nc.sync.dma_start(out[:, :], out_sb[:, :])
    nc.finalize()
    return nc


_NC_CACHE = None


def _get_nc():
    global _NC_CACHE
    if _NC_CACHE is None:
        _NC_CACHE = _build()
    return _NC_CACHE


def make_in_maps(x_input, targets, w1, b1, w2, b2):
    x_input = np.ascontiguousarray(x_input, dtype=np.float32)
    targets = np.ascontiguousarray(targets, dtype=np.float32)
    w1 = np.ascontiguousarray(w1, dtype=np.float32)
    b1 = np.ascontiguousarray(b1, dtype=np.float32)
    w2 = np.ascontiguousarray(w2, dtype=np.float32)
    b2 = np.ascontiguousarray(b2, dtype=np.float32)
    c2n = (2.0 * w1.astype(np.float64) ** 2 * w2.astype(np.float64)).astype(np.float32)
    pidx = np.arange(P)

    def signs_col(k):
        return np.where((pidx & (k // W)) == 0, 1.0, -1.0).astype(np.float32)

    signs = np.stack([signs_col(k) for k in SGN_KS], axis=1).astype(np.float32)
    identity = np.eye(P, dtype=np.float32)
    shiftu = np.eye(P, P, -1, dtype=np.float32)  # auxU[m] = s[m+1]
    shiftd = np.eye(P, P, 1, dtype=np.float32)   # auxD[m] = s[m-1]
    in_maps = []
    ones_row = np.ones(SHARD, np.float32)
    for c in range(NCORES):
        ts = targets[c * SHARD : (c + 1) * SHARD].reshape(NCH, P).T
        xflat = np.concatenate([x_input[c * SHARD : (c + 1) * SHARD], ones_row])[None, :]
        in_maps.append(
            {
                "x_full": x_input,
                "t_shard": np.ascontiguousarray(ts),
                "w1": w1,
                "b1": b1,
                "w2": w2,
                "b2": b2,
                "c2n": c2n,
                "x_flat": np.ascontiguousarray(xflat),
                "signs": signs,
                "ident": identity,
                "shiftu": shiftu,
                "shiftd": shiftd,
            }
        )
    return in_maps


def kernel(x_input, targets, w1, b1, w2, b2, **_ignored):
    in_maps = make_in_maps(x_input, targets, w1, b1, w2, b2)
    nc = _get_nc()
    res = run_bass_kernel_spmd(nc, in_maps, core_ids=list(range(NCORES)))
    outs = [r["out"] for r in res.results]

    dsum = outs[0][:, :W].astype(np.float64).ravel()  # d1 + d2 per point
    dens = 1.0 / (dsum / 3.0 + 2.0 * EPS)
    sse = sum(o[:, W].astype(np.float64).sum() for o in outs)
    d2sq = sum(o[:, W + 1].astype(np.float64).sum() for o in outs)

    mse = sse / N
    mean_densn = (dens.sum() / N) / (dens.max() + EPS)
    penalty = 0.01 * (1.0 + 0.1 * mean_densn) * (d2sq / N)
    total = mse + penalty
    return np.array([total, mse, penalty], dtype=np.float32)
